# revision 1
# baseline (speedup 1.0000x reference)
"""DGCNN-style edge-conv block (KNN graph + dense conv stack) on 8 trn2 cores.

Strategy (data-parallel over batch, one batch element per core):
  scores   = -||xi - xj||^2 via one fp32 PE matmul with [2x; -1] x [x; x^2]
             contraction + per-partition -xx_i bias on the ACT evacuation.
  top-16   = DVE max8 / max_index / match_replace / max8 / max_index chain
             per 128-row tile (exact, fp32).
  gather   = P^T table (P = W1a @ x, 64 ch fp32 = 256B rows) in DRAM,
             gathered per 8192-edge super-chunk with gpsimd dma_gather
             (mlp ucode library, single_packet=False).
  edge MLP = A = relu(P_j + T_n), B2 = relu(W2a A + R_n),
             C3 = W3a A + W3c B2 + S_n, with T/R/S = per-node tables from
             small matmuls; per-edge convs run as f32r block-diag matmuls
             on PE with 2k-stacked PE transposes. The R bias is folded into
             the conv2 PSUM group as an identity x R-broadcast f32r matmul
             (plain-matmul accumulation is safe; mixing with is_transpose
             matmuls in one group crashes the device).
  output   = channel-concat [max_k A; x; max_k B2; max_k C3].

Schedule: all 16 row-tiles' scores+topk are emitted first so the DVE top-k
chain (the dominant engine cost, ~175us) streams back-to-back; each
super-chunk's gather/transpose/conv/max stages trail behind it on DMA, PE
and ACT as soon as its 4 index tiles are ready. k-maxes run as contiguous
tensor_tensor trees (8->4->2->1 over k-pairs) rather than strided reduces.
Cost-model estimate: ~320us/core (DVE busy ~257us is the floor: 5 full-width
streaming passes per tile for exact top-16 is what the DVE ISA costs).
"""

import numpy as np

import bass_rust
import concourse.bass as bass
import concourse.bass_isa as bass_isa
import concourse.mybir as mybir
from concourse.bass_types import AP
from concourse.tile import TileContext
from concourse.bass_utils import run_bass_kernel_spmd

F32 = mybir.dt.float32
F32R = mybir.dt.float32r
U16 = mybir.dt.uint16
I16 = mybir.dt.int16

B, C, N, K, G = 8, 64, 2048, 16, 64
NT = 16          # 128-row tiles
NSC = 4          # super-chunks
NBL = 4          # nblocks per super-chunk
RELU = mybir.ActivationFunctionType.Relu
COPY = mybir.ActivationFunctionType.Copy
SQUARE = mybir.ActivationFunctionType.Square
ADD = mybir.AluOpType.add
MAX = mybir.AluOpType.max

_nop_ctr = [0]


def _split_all_waits(nc, max_waits=1):
    # This walrus build rejects >1 sync-wait on several CTRL structs; hoist
    # extras onto single-wait NOPs placed just before the instruction.
    for fn in nc.m.functions:
        for bb in fn.blocks:
            out = []
            for ins in bb.instructions:
                si = ins.sync_info
                if si is not None and si.on_wait is not None and len(si.on_wait) > max_waits:
                    waits = list(si.on_wait)
                    for w in waits[:-max_waits]:
                        _nop_ctr[0] += 1
                        nop = mybir.InstNoOp(name=f"waitnop-{_nop_ctr[0]}", ins=[], outs=[])
                        nop.engine = ins.engine
                        nop.sync_info = bass_rust.SyncInfo(on_wait=[w], on_update=[])
                        out.append(nop)
                        nc.register_instruction(nop, overwrite=True)
                    si.on_wait = waits[-max_waits:]
                out.append(ins)
            bb.instructions = out


def _insert_gpsimd_library_load(nc, lib_index=3):
    # InstDMAGatherAnt needs the 'mlp' GPSIMD ucode library; raw Bass+Tile
    # skips Bacc's insert_library_loads, so prepend the reload by hand.
    ins = bass_isa.InstPseudoReloadLibraryIndex(
        name="libload-manual", ins=[], outs=[], lib_index=lib_index
    )
    ins.engine = mybir.EngineType.Pool
    nc.register_instruction(ins, overwrite=True)
    bb0 = nc.m.functions[0].blocks[0]
    bb0.instructions = [ins] + list(bb0.instructions)
    mybir.codegen_inst_isa_subclasses(nc)


def build():
    nc = bass.Bass("TRN2", debug=False, num_devices=8)

    x_in = nc.dram_tensor("x", [C, N], F32, kind="ExternalInput")
    WLTP = nc.dram_tensor("WLTP", [64, 64], F32, kind="ExternalInput")    # W1a.T
    WLT = nc.dram_tensor("WLT", [65, 64], F32, kind="ExternalInput")      # [(W1b-W1a).T; b1]
    WLR = nc.dram_tensor("WLR", [65, 64], F32, kind="ExternalInput")      # [W2b.T; b2]
    WLS = nc.dram_tensor("WLS", [65, 64], F32, kind="ExternalInput")      # [W3b.T; b3]
    W2BLK = nc.dram_tensor("W2BLK", [128, 128], F32R, kind="ExternalInput")
    W3ABLK = nc.dram_tensor("W3ABLK", [128, 128], F32R, kind="ExternalInput")
    W3CBLK = nc.dram_tensor("W3CBLK", [128, 128], F32R, kind="ExternalInput")
    EYE = nc.dram_tensor("EYE", [128, 128], F32, kind="ExternalInput")
    EYER = nc.dram_tensor("EYER", [128, 128], F32R, kind="ExternalInput")
    Y = nc.dram_tensor("y", [C + 3 * G, N], F32, kind="ExternalOutput")

    PT_D = nc.dram_tensor("PT_D", [N, C], F32, kind="Internal")
    IDXD = nc.dram_tensor("IDXD", [N * K], I16, kind="Internal")

    with TileContext(nc) as tc:
        with tc.tile_pool(name="const", bufs=1) as cp, \
             tc.tile_pool(name="work", bufs=2) as wp, \
             tc.tile_pool(name="chunk", bufs=1) as kp, \
             tc.tile_pool(name="gat", bufs=2) as gp, \
             tc.tile_pool(name="ps2", bufs=2, space="PSUM") as pps2, \
             tc.tile_pool(name="ps1", bufs=2, space="PSUM") as pps1:

            # ---------------- setup ----------------
            X65 = cp.tile([65, N], F32)
            RHSB = cp.tile([128, N], F32)
            LHSB = cp.tile([128, N], F32)
            NXX = cp.tile([128, NT], F32)
            PC = cp.tile([64, N], F32)
            TSTK = cp.tile([128, N], F32)
            RSTK = cp.tile([128, N], F32R)
            SCt = cp.tile([64, N], F32)
            PTS = cp.tile([128, NT * 64], F32)
            IDXALL = cp.tile([128, NT * K], U16)
            EYEt = cp.tile([128, 128], F32)
            EYERt = cp.tile([128, 128], F32R)
            ONES64 = cp.tile([64, 1], F32)
            wltp = cp.tile([64, 64], F32)
            wlt = cp.tile([65, 64], F32)
            wlr = cp.tile([65, 64], F32)
            wls = cp.tile([65, 64], F32)
            w2b = cp.tile([128, 128], F32R)
            w3a = cp.tile([128, 128], F32R)
            w3c = cp.tile([128, 128], F32R)

            nc.sync.dma_start(out=X65[0:64, :], in_=x_in[:, :])
            nc.sync.dma_start(out=RHSB[0:64, :], in_=x_in[:, :])
            nc.sync.dma_start(out=EYEt[:, :], in_=EYE[:, :])
            nc.sync.dma_start(out=EYERt[:, :], in_=EYER[:, :])
            nc.sync.dma_start(out=wltp[:, :], in_=WLTP[:, :])
            nc.sync.dma_start(out=wlt[:, :], in_=WLT[:, :])
            nc.sync.dma_start(out=wlr[:, :], in_=WLR[:, :])
            nc.sync.dma_start(out=wls[:, :], in_=WLS[:, :])
            nc.sync.dma_start(out=w2b[:, :], in_=W2BLK[:, :])
            nc.sync.dma_start(out=w3a[:, :], in_=W3ABLK[:, :])
            nc.sync.dma_start(out=w3c[:, :], in_=W3CBLK[:, :])
            nc.vector.memset(X65[64:65, :], 1.0)
            nc.vector.memset(LHSB[64:128, :], -1.0)
            nc.vector.memset(ONES64[:, :], 1.0)

            X2 = cp.tile([64, N], F32)
            nc.scalar.activation(X2[:, :], X65[0:64, :], SQUARE)
            nc.scalar.activation(RHSB[64:128, :], X2[:, :], COPY)
            nc.scalar.activation(LHSB[0:64, :], X65[0:64, :], COPY, scale=2.0)

            # xx per node (128, 16), negated
            ps_xx = pps1.tile([128, NT], F32, tag="u2")
            for rt in range(NT):
                nc.tensor.matmul(ps_xx[:, rt:rt + 1],
                                 X2[:, rt * 128:(rt + 1) * 128],
                                 ONES64[:, :], start=True, stop=True)
            nc.scalar.activation(NXX[:, :], ps_xx[:, :], COPY, scale=-1.0)

            # P (c-layout), T/R stacked, S  — small fp32 matmuls
            for u in range(4):
                sl = slice(u * 512, (u + 1) * 512)
                p1 = pps1.tile([64, 512], F32, tag="u2")
                nc.tensor.matmul(p1[:, :], wltp[:, :], RHSB[0:64, sl], start=True, stop=True)
                nc.scalar.activation(PC[:, sl], p1[:, :], COPY)
                p2 = pps1.tile([64, 512], F32, tag="u2")
                nc.tensor.matmul(p2[:, :], wlt[:, :], X65[:, sl], start=True, stop=True)
                nc.scalar.activation(TSTK[0:64, sl], p2[:, :], COPY)
                p3 = pps1.tile([64, 512], F32, tag="u2")
                nc.tensor.matmul(p3[:, :], wlr[:, :], X65[:, sl], start=True, stop=True)
                nc.scalar.activation(RSTK[0:64, sl], p3[:, :], COPY)
                p4 = pps1.tile([64, 512], F32, tag="u2")
                nc.tensor.matmul(p4[:, :], wls[:, :], X65[:, sl], start=True, stop=True)
                nc.scalar.activation(SCt[:, sl], p4[:, :], COPY)
            nc.scalar.activation(TSTK[64:128, :], TSTK[0:64, :], COPY)
            nc.scalar.activation(RSTK[64:128, :], RSTK[0:64, :], COPY)

            # P^T table -> DRAM
            for rt in range(NT):
                pt = pps1.tile([128, 64], F32, tag="u2")
                nc.tensor.transpose(pt[:, :], PC[:, rt * 128:(rt + 1) * 128],
                                    EYEt[0:64, 0:64])
                nc.scalar.activation(PTS[:, rt * 64:(rt + 1) * 64], pt[:, :], COPY)
            nc.sync.dma_start(
                out=AP(PT_D, 0, [[64, 128], [8192, NT], [1, 64]]),
                in_=PTS[:, :].rearrange("p (a b) -> p a b", a=NT),
            )

            # x passthrough output rows 64:128
            nc.sync.dma_start(out=Y[64:128, :], in_=X65[0:64, :])

            # ---------------- scores + topk for all row tiles ----------------
            for rt in range(NT):
                if True:
                    SCORES = wp.tile([128, N], F32, tag="scores")
                    for u in range(2):
                        pss = pps2.tile([128, 1024], F32, tag="score")
                        for h in range(2):
                            nc.tensor.matmul(pss[:, h * 512:(h + 1) * 512],
                                             LHSB[:, rt * 128:(rt + 1) * 128],
                                             RHSB[:, u * 1024 + h * 512:u * 1024 + (h + 1) * 512],
                                             start=True, stop=True)
                        nc.scalar.activation(SCORES[:, u * 1024:(u + 1) * 1024],
                                             pss[:, :],
                                             mybir.ActivationFunctionType.Identity,
                                             bias=NXX[:, rt:rt + 1])
                    t8a = wp.tile([128, 8], F32, tag="t8a")
                    t8b = wp.tile([128, 8], F32, tag="t8b")
                    nc.vector.max(out=t8a[:, :], in_=SCORES[:, :])
                    nc.vector.max_index(out=IDXALL[:, rt * K:rt * K + 8],
                                        in_max=t8a[:, :], in_values=SCORES[:, :])
                    nc.vector.match_replace(out=SCORES[:, :], in_to_replace=t8a[:, :],
                                            in_values=SCORES[:, :], imm_value=-3.0e38)
                    nc.vector.max(out=t8b[:, :], in_=SCORES[:, :])
                    nc.vector.max_index(out=IDXALL[:, rt * K + 8:rt * K + 16],
                                        in_max=t8b[:, :], in_values=SCORES[:, :])

            # ---------------- per super-chunk ----------------
            for sc in range(NSC):
                # idx -> DRAM (e = nbl*2048 + k*128 + r order) -> wrapped read
                # IDXD layout: addr = r*64 + nbl*16 + k (contiguous per SBUF
                # partition on the write; the read AP compensates). Avoids
                # 2-byte-per-descriptor DMA shatter in both directions.
                nc.sync.dma_start(
                    out=AP(IDXD, sc * 8192, [[64, 128], [16, NBL], [1, K]]),
                    in_=IDXALL[:, sc * 64:(sc + 1) * 64].bitcast(I16)
                        .rearrange("p (a b) -> p a b", a=NBL),
                )
                idxt = gp.tile([128, 512], I16, tag="idxt")
                # idxt[p, s] with s = nbl*128 + k*8 + rhi <- addr p*64 + rhi*1024 + nbl*16 + k
                src_w = AP(IDXD, sc * 8192, [[64, 16], [16, NBL], [1, K], [1024, 8]])
                nc.sync.dma_start(out=idxt[0:16, :], in_=src_w)
                for g in range(1, 8):
                    nc.sync.dma_start(out=idxt[16 * g:16 * (g + 1), :],
                                      in_=idxt[0:16, :])

                PG = gp.tile([128, 64, 64], F32, tag="pg")
                nc.gpsimd.dma_gather(
                    out_ap=PG[:, :, :], in_ap=PT_D.ap(), idxs_ap=idxt[:, :],
                    num_idxs=8192, num_idxs_reg=8192, elem_size=64,
                    single_packet=False,
                )

                AC = kp.tile([128, NBL, 8, 128], F32R, tag="ac")
                B2C = kp.tile([128, NBL, 8, 128], F32R, tag="b2c")
                C3C = kp.tile([128, NBL, 8, 128], F32, tag="c3c")

                for bl in range(NBL):
                    g = sc * NBL + bl
                    tb = TSTK[:, g * 128:(g + 1) * 128].unsqueeze(1).broadcast_to([128, 4, 128])
                    rb = RSTK[:, g * 128:(g + 1) * 128].unsqueeze(1).broadcast_to([128, 4, 128])
                    for q in range(2):
                        # transposes: 4 kp blocks -> psum (128, 512)
                        psa = pps2.tile([128, 512], F32, tag="a")
                        for kk in range(4):
                            kpi = q * 4 + kk
                            blk = PG[:, bl * 16 + 2 * kpi:bl * 16 + 2 * kpi + 2, :]
                            nc.tensor.transpose(psa[:, kk * 128:(kk + 1) * 128],
                                                blk, EYEt[:, :])
                        sa = wp.tile([128, 512], F32, tag="sa")
                        nc.vector.scalar_tensor_tensor(out=sa[:, :], in0=psa[:, :],
                                                       scalar=0.0, in1=tb,
                                                       op0=ADD, op1=ADD)
                        nc.scalar.activation(AC[:, bl, 4 * q:4 * q + 4, :], sa[:, :], RELU)

                        # conv2 (+R folded in as an identity-matmul accumulate)
                        ps2t = pps1.tile([128, 512], F32, tag="u2")
                        nc.tensor.matmul(ps2t[:, :], w2b[:, :],
                                         AC[:, bl, 4 * q:4 * q + 4, :],
                                         start=True, stop=False,
                                         skip_group_check=True)
                        nc.tensor.matmul(ps2t[:, :].rearrange("p (a b) -> p a b", a=4),
                                         EYERt[:, :], rb,
                                         start=False, stop=True,
                                         skip_group_check=True)
                        nc.scalar.activation(B2C[:, bl, 4 * q:4 * q + 4, :], ps2t[:, :], RELU)

                        # conv3 (accumulate two matmuls)
                        ps3t = pps1.tile([128, 512], F32, tag="u2")
                        nc.tensor.matmul(ps3t[:, :], w3a[:, :],
                                         AC[:, bl, 4 * q:4 * q + 4, :],
                                         start=True, stop=False)
                        nc.tensor.matmul(ps3t[:, :], w3c[:, :],
                                         B2C[:, bl, 4 * q:4 * q + 4, :],
                                         start=False, stop=True)
                        nc.scalar.activation(C3C[:, bl, 4 * q:4 * q + 4, :], ps3t[:, :], COPY)

                # maxes over k (kp in free dim, then k-parity across the
                # 64-partition halves), then DMA the chunk's output columns.
                for (src, row0, add_s) in ((AC, 0, False), (B2C, 2 * G, False), (C3C, 3 * G, True)):
                    sr = src[:, :, :, :].bitcast(F32)
                    m1 = kp.tile([128, NBL, 4, 128], F32, tag="m1")
                    nc.vector.tensor_tensor(out=m1[:, :, :, :], in0=sr[:, :, 0:4, :],
                                            in1=sr[:, :, 4:8, :], op=MAX)
                    m2 = kp.tile([128, NBL, 2, 128], F32, tag="m2")
                    nc.vector.tensor_tensor(out=m2[:, :, :, :], in0=m1[:, :, 0:2, :],
                                            in1=m1[:, :, 2:4, :], op=MAX)
                    red = kp.tile([128, NBL, 128], F32, tag=f"red{row0}")
                    nc.vector.tensor_tensor(out=red[:, :, :], in0=m2[:, :, 0, :],
                                            in1=m2[:, :, 1, :], op=MAX)
                    hi = kp.tile([64, NBL * 128], F32, tag=f"hi{row0}")
                    nc.scalar.activation(hi[:, :],
                                         red[64:128, :, :].rearrange("p a n -> p (a n)"),
                                         COPY)
                    om = kp.tile([64, NBL * 128], F32, tag=f"om{row0}")
                    nc.vector.tensor_tensor(out=om[:, :],
                                            in0=red[0:64, :, :].rearrange("p a n -> p (a n)"),
                                            in1=hi[:, :], op=MAX)
                    if add_s:
                        nc.vector.tensor_tensor(out=om[:, :], in0=om[:, :],
                                                in1=SCt[:, sc * 512:(sc + 1) * 512],
                                                op=ADD)
                    nc.sync.dma_start(out=Y[row0 if row0 else 0:(row0 if row0 else 0) + 64,
                                            sc * 512:(sc + 1) * 512],
                                      in_=om[:, :])

    _split_all_waits(nc)
    _insert_gpsimd_library_load(nc, 3)
    return nc


def _prep_weights(W1, b1, W2, b2, W3, b3):
    W1 = np.asarray(W1, np.float32); W2 = np.asarray(W2, np.float32)
    W3 = np.asarray(W3, np.float32)
    b1 = np.asarray(b1, np.float32); b2 = np.asarray(b2, np.float32)
    b3 = np.asarray(b3, np.float32)
    W1a, W1b = W1[:, :64], W1[:, 64:]
    W2a, W2b = W2[:, :64], W2[:, 64:]
    W3a, W3b, W3c = W3[:, :64], W3[:, 64:128], W3[:, 128:]

    def blk(w):
        z = np.zeros((128, 128), np.float32)
        z[0:64, 0:64] = w.T
        z[64:128, 64:128] = w.T
        return z

    return {
        "WLTP": np.ascontiguousarray(W1a.T),
        "WLT": np.ascontiguousarray(np.vstack([(W1b - W1a).T, b1[None, :]])),
        "WLR": np.ascontiguousarray(np.vstack([W2b.T, b2[None, :]])),
        "WLS": np.ascontiguousarray(np.vstack([W3b.T, b3[None, :]])),
        "W2BLK": blk(W2a),
        "W3ABLK": blk(W3a),
        "W3CBLK": blk(W3c),
        "EYE": np.eye(128, dtype=np.float32),
        "EYER": np.eye(128, dtype=np.float32),
    }


_NC = None


def kernel(x, W1, b1, W2, b2, W3, b3):
    global _NC
    if _NC is None:
        _NC = build()
    x = np.asarray(x, np.float32)
    w = _prep_weights(W1, b1, W2, b2, W3, b3)
    in_maps = [{"x": np.ascontiguousarray(x[b]), **w} for b in range(B)]
    res = run_bass_kernel_spmd(_NC, in_maps, core_ids=list(range(B)))
    return np.stack([res.results[b]["y"] for b in range(B)], axis=0)

